# revision 11
# baseline (speedup 1.0000x reference)
"""Multi-head attention (B=2, N=2048, C=768, H=12) on 8 TRN2 NeuronCores.

Sharding: core c = 4*b + g handles batch b (data parallel) and heads
3g..3g+2 (tensor parallel on H). Each core computes its 3 heads end-to-end
plus the partial projection with its 192 rows of w_proj; the host sums the
4 partials per batch (f32) and adds b_proj. No cross-device communication.

Engine assignment (per core):
  PE    : qkv projections, scores (64-row-tile pairs), attn@v_aug, proj.
          Emitted as one near-gapless stream so the HAM latches 2.4 GHz:
          qk_h1/qk_h2/v/proj matmuls are interleaved as filler between the
          ACT-paced attention chunks.
  ACT   : exp only (~1.04us per [128,1024] PSUM tile -> the pace setter).
  DVE   : PSUM evacuation copies (qk halves, v), reciprocal, final
          normalization multiplies.
  DMA   : input loads, q<->k partition swaps, oa->SBUF evacuation,
          reciprocal partition-broadcast roundtrip, proj PSUM->DRAM in f32.
"""

from collections import deque

import ml_dtypes
import numpy as np

import concourse.bass as bass
import concourse.mybir as mybir
import concourse.tile as tile
from concourse import bacc
from concourse.bass_utils import run_bass_kernel_spmd

F32 = mybir.dt.float32
F32R = mybir.dt.float32r
BF16 = mybir.dt.bfloat16
EXP = mybir.ActivationFunctionType.Exp
MULT = mybir.AluOpType.mult

B, N, C = 2, 2048, 768
H = 12
D = 64
HPC = 3  # heads per core
KC = 6  # contraction chunks of 128 over C
NB = 1024  # n-block for attention stage
MC = N // 128  # 16 m-chunks (context)
NCH = N // 128  # 16 row chunks of output
SCALE = D ** -0.5
LAG = 2  # oa trails exp by this many m-chunks so oa never stalls the PE

_NC_CACHE = None


def build_nc():
    nc = bacc.Bacc("TRN2", target_bir_lowering=False, debug=False, num_devices=8)
    xt = nc.declare_dram_parameter("xt", [C, N], BF16, isOutput=False)
    wqk = nc.declare_dram_parameter("wqk", [C, HPC * 128], BF16, isOutput=False)
    wv = nc.declare_dram_parameter("wv", [C, HPC * D], BF16, isOutput=False)
    wp = nc.declare_dram_parameter("wp", [HPC * D, C], BF16, isOutput=False)
    out = nc.declare_dram_parameter("out", [N, C], BF16, isOutput=True)

    with tile.TileContext(nc) as tc:
        with (
            tc.tile_pool(name="sb", bufs=1) as sb,
            tc.tile_pool(name="ps", bufs=1, space="PSUM") as ps,
            tc.tile_pool(name="drp", bufs=2, space="DRAM") as drp,
        ):
            # ---- input loads --------------------------------------------
            xtb = sb.tile([128, KC * N], BF16, tag="xtb")
            wqkb = sb.tile([128, KC * HPC * 128], BF16, tag="wqkb")
            wvb = sb.tile([128, KC * HPC * D], BF16, tag="wvb")
            for kc in range(KC):
                nc.sync.dma_start(
                    wqkb[:, kc * HPC * 128 : (kc + 1) * HPC * 128],
                    wqk[kc * 128 : (kc + 1) * 128, :],
                )
                nc.sync.dma_start(
                    wvb[:, kc * HPC * D : (kc + 1) * HPC * D],
                    wv[kc * 128 : (kc + 1) * 128, :],
                )
                nc.sync.dma_start(
                    xtb[:, kc * N : (kc + 1) * N], xt[kc * 128 : (kc + 1) * 128, :]
                )
            wp01 = sb.tile([128, C], BF16, tag="wp01")
            nc.sync.dma_start(wp01[:], wp[0:128, :])
            wp2 = sb.tile([64, C], BF16, tag="wp2")
            nc.sync.dma_start(wp2[:], wp[128 : HPC * D, :])

            junk = sb.tile([128, 512], BF16, tag="junk")
            nc.vector.memset(junk[:], 1.0)

            # persistent activations
            ones_f = sb.tile([128, MC], F32, tag="ones_f")
            nc.vector.memset(ones_f[:], 1.0)
            v_sb = sb.tile([128, HPC * MC * 65], F32R, tag="v")
            v4 = v_sb.rearrange("p (h m w) -> p h m w", h=HPC, m=MC)
            for h in range(HPC):
                nc.vector.tensor_copy(v4[:, h, :, 64], ones_f[:, :])

            qk_sb = [
                sb.tile([128, N], BF16, tag=f"qk{h}", name=f"qk{h}") for h in range(HPC)
            ]
            kq_sb = [
                sb.tile([128, N], BF16, tag=f"kq{h}", name=f"kq{h}") for h in range(HPC)
            ]
            stk = sb.tile([128, N], BF16, tag="stk")
            outT1 = sb.tile([64, N], BF16, tag="outT1")
            outT2 = sb.tile([64, N], BF16, tag="outT2")

            # ---- PE warmup: ramp HAM while inputs stream in -------------
            for i in range(16):
                psw = ps.tile([128, NB], F32, tag="sc", bufs=3, name=f"junk{i}")
                nc.tensor.matmul(
                    psw[:, 0:512], junk[:, 0:128], junk[:], start=True, stop=True
                )

            # ---- qk for one head: 2 psum halves, copy + swap ------------
            def emit_qk_half(h, half, psq):
                hb = half * 1024
                for kc in range(KC):
                    for s in range(2):
                        nc.tensor.matmul(
                            psq[:, s * 512 : (s + 1) * 512],
                            wqkb[
                                :,
                                kc * HPC * 128 + h * 128 : kc * HPC * 128 + (h + 1) * 128,
                            ],
                            xtb[:, kc * N + hb + s * 512 : kc * N + hb + (s + 1) * 512],
                            start=(kc == 0),
                            stop=(kc == KC - 1),
                        )

            def emit_qk_tail(h, half, psq):
                hb = half * 1024
                nc.vector.tensor_copy(qk_sb[h][:, hb : hb + 1024], psq[:])
                nc.sync.dma_start(
                    kq_sb[h][0:64, hb : hb + 1024], qk_sb[h][64:128, hb : hb + 1024]
                )
                nc.sync.dma_start(
                    kq_sb[h][64:128, hb : hb + 1024], qk_sb[h][0:64, hb : hb + 1024]
                )

            # lead-in: head 0 qk directly (PE gapless)
            for half in range(2):
                psq = ps.tile([128, NB], F32, tag="sc", bufs=3, name=f"qk0h{half}")
                emit_qk_half(0, half, psq)
                emit_qk_tail(0, half, psq)

            # ---- background PE work items -------------------------------
            bg = deque()

            def v_item(m):
                def emit():
                    psv = ps.tile([128, NB], F32, tag="sc", bufs=3, name=f"v{m}")
                    for kc in range(KC):
                        nc.tensor.matmul(
                            psv[:, 0 : HPC * D],
                            xtb[:, kc * N + m * 128 : kc * N + (m + 1) * 128],
                            wvb[:, kc * HPC * D : (kc + 1) * HPC * D],
                            start=(kc == 0),
                            stop=(kc == KC - 1),
                        )
                    nc.vector.tensor_copy(
                        v4[:, :, m, 0:64],
                        psv[:, 0 : HPC * D].rearrange("p (h d) -> p h d", h=HPC),
                    )

                return emit

            def qk_item(h, half, state, step):
                # step 0..KC-1: one kc contraction step (2 matmuls, ~427ns);
                # step KC: evacuation copy + partition swap (no PE work)
                def emit():
                    hb = half * 1024
                    if step == 0:
                        state["ps"] = ps.tile(
                            [128, NB], F32, tag="sc", bufs=3, name=f"qk{h}h{half}"
                        )
                    psq = state["ps"]
                    if step < KC:
                        kc = step
                        for s in range(2):
                            nc.tensor.matmul(
                                psq[:, s * 512 : (s + 1) * 512],
                                wqkb[
                                    :,
                                    kc * HPC * 128
                                    + h * 128 : kc * HPC * 128
                                    + (h + 1) * 128,
                                ],
                                xtb[
                                    :,
                                    kc * N + hb + s * 512 : kc * N + hb + (s + 1) * 512,
                                ],
                                start=(kc == 0),
                                stop=(kc == KC - 1),
                            )
                    else:
                        emit_qk_tail(h, half, psq)

                return emit

            def proj_item(k):
                def emit():
                    pp = ps.tile([128, NB], F32, tag="sc", bufs=3, name=f"pp{k}")
                    for sw, w in ((0, 512), (512, 256)):
                        nc.tensor.matmul(
                            pp[:, sw : sw + w],
                            stk[:, k * 128 : (k + 1) * 128],
                            wp01[:, sw : sw + w],
                            start=True,
                            stop=False,
                        )
                    for sw, w in ((0, 512), (512, 256)):
                        nc.tensor.matmul(
                            pp[:, sw : sw + w],
                            outT2[0:64, k * 128 : (k + 1) * 128],
                            wp2[:, sw : sw + w],
                            start=False,
                            stop=True,
                        )
                    ob = sb.tile([128, C], BF16, tag="ob", bufs=3, name=f"ob{k}")
                    nc.vector.tensor_copy(ob[:], pp[:, 0:C])
                    nc.sync.dma_start(out[k * 128 : (k + 1) * 128, :], ob[:])

                return emit

            for m in range(MC):
                bg.append(v_item(m))
            for h in (1, 2):
                for half in range(2):
                    state = {"ps": None}
                    for step in range(KC + 1):
                        bg.append(qk_item(h, half, state, step))

            pad_idx = [0]

            def pump(n, pad=False):
                for _ in range(n):
                    if bg:
                        bg.popleft()()
                    elif pad:
                        # keep the PE stream gapless so the HAM stays latched
                        psw = ps.tile(
                            [128, NB], F32, tag="sc", bufs=3,
                            name=f"pad{pad_idx[0]}",
                        )
                        pad_idx[0] += 1
                        for s in range(2):
                            nc.tensor.matmul(
                                psw[:, s * 512 : (s + 1) * 512],
                                junk[:, 0:128],
                                junk[:],
                                start=True,
                                stop=True,
                            )

            # ---- attention: head outer, nb inner ------------------------
            for h in range(HPC):
                for nb in range(N // NB):
                    ex_tiles = {}
                    oa = None

                    def emit_oa(m):
                        exm = ex_tiles.pop(m)
                        for s in range(2):
                            nc.tensor.matmul(
                                oa[:, s * 512 : (s + 1) * 512],
                                v4[:, h, m, :],
                                exm[:, s * 512 : (s + 1) * 512],
                                start=(m == 0),
                                stop=(m == MC - 1),
                            )

                    for m in range(MC):
                        sc = ps.tile([128, NB], F32, tag="sc", bufs=3, name=f"sc{nb}_{h}_{m}")
                        nc.tensor.matmul(
                            sc[:, 0:512],
                            kq_sb[h][0:64, m * 128 : (m + 1) * 128],
                            qk_sb[h][0:64, nb * NB : nb * NB + 512],
                            start=True,
                            stop=True,
                            tile_position=(0, 0),
                        )
                        nc.tensor.matmul(
                            sc[:, 512:1024],
                            qk_sb[h][64:128, m * 128 : (m + 1) * 128],
                            kq_sb[h][64:128, nb * NB + 512 : nb * NB + 1024],
                            start=True,
                            stop=True,
                            tile_position=(64, 0),
                        )
                        ex = sb.tile([128, NB], F32R, tag="ex", bufs=6)
                        nc.scalar.activation(ex[:], sc[:], EXP, scale=SCALE)
                        ex_tiles[m] = ex
                        pump(1, pad=True)
                        if m == LAG:
                            oa = ps.tile([65, NB], F32, tag="oa", bufs=1)
                        if m >= LAG:
                            emit_oa(m - LAG)
                    for m in range(MC - LAG, MC):
                        emit_oa(m)
                        pump(1)

                    # epilogue: normalization (no PE involvement)
                    cs = sb.tile([1, NB], F32, tag="cs", bufs=2)
                    nc.vector.tensor_copy(cs[:], oa[64:65, :])
                    osb = sb.tile([64, NB], F32, tag="osb", bufs=3)
                    nc.vector.tensor_copy(osb[:], oa[0:64, :])
                    rf = sb.tile([1, NB], F32, tag="rf", bufs=2)
                    nc.vector.reciprocal_approx_fast(out=rf[:], in_=cs[:])
                    rfd = drp.tile([1, NB], F32, tag="rfd", bufs=2)
                    nc.sync.dma_start(rfd[:], rf[:])
                    rbs = sb.tile([64, NB], F32, tag="rbs", bufs=2)
                    nc.sync.dma_start(rbs[:], rfd[:].partition_broadcast(64))
                    if h == 0:
                        mdst = stk[0:64, nb * NB : (nb + 1) * NB]
                    elif h == 1:
                        mdst = outT1[0:64, nb * NB : (nb + 1) * NB]
                    else:
                        mdst = outT2[0:64, nb * NB : (nb + 1) * NB]
                    nc.vector.tensor_tensor(out=mdst, in0=osb[:], in1=rbs[:], op=MULT)
                    if h == 1:
                        nc.sync.dma_start(
                            stk[64:128, nb * NB : (nb + 1) * NB],
                            outT1[0:64, nb * NB : (nb + 1) * NB],
                        )

                    # after the last head finishes a token range, its proj
                    # chunks become eligible PE filler
                    if h == HPC - 1:
                        for k in range(nb * 8, nb * 8 + 8):
                            bg.append(proj_item(k))

            pump(len(bg))

    nc.compile()
    return nc


def get_nc():
    global _NC_CACHE
    if _NC_CACHE is None:
        _NC_CACHE = build_nc()
    return _NC_CACHE


def make_in_maps(x, w_qkv, w_proj):
    """Shard inputs for the 8 cores: core c = 4*b + g."""
    in_maps = []
    for c in range(8):
        b, g = divmod(c, 4)
        heads = [3 * g + h for h in range(HPC)]
        xt = np.ascontiguousarray(x[b].T).astype(ml_dtypes.bfloat16)
        wqk = np.empty((C, HPC * 128), dtype=ml_dtypes.bfloat16)
        wv = np.empty((C, HPC * D), dtype=ml_dtypes.bfloat16)
        for i, hh in enumerate(heads):
            wqk[:, i * 128 : i * 128 + 64] = w_qkv[:, hh * D : (hh + 1) * D]
            wqk[:, i * 128 + 64 : i * 128 + 128] = w_qkv[
                :, C + hh * D : C + (hh + 1) * D
            ]
            wv[:, i * D : (i + 1) * D] = w_qkv[:, 2 * C + hh * D : 2 * C + (hh + 1) * D]
        wp = np.ascontiguousarray(w_proj[g * HPC * D : (g + 1) * HPC * D, :]).astype(
            ml_dtypes.bfloat16
        )
        in_maps.append(
            {"xt": xt, "wqk": np.ascontiguousarray(wqk), "wv": wv, "wp": wp}
        )
    return in_maps


def run(x, w_qkv, w_proj, b_proj, trace=False):
    nc = get_nc()
    in_maps = make_in_maps(x, w_qkv, w_proj)
    res = run_bass_kernel_spmd(nc, in_maps, core_ids=list(range(8)), trace=trace)
    out = np.empty((B, N, C), dtype=np.float32)
    for b in range(B):
        acc = res.results[4 * b]["out"].astype(np.float32)
        for g in range(1, 4):
            acc = acc + res.results[4 * b + g]["out"]
        out[b] = acc + b_proj[None, :].astype(np.float32)
    return out, res


def kernel(x, w_qkv, w_proj, b_proj):
    out, _ = run(
        np.asarray(x), np.asarray(w_qkv), np.asarray(w_proj), np.asarray(b_proj)
    )
    return out


# revision 17
# speedup vs baseline: 1.2115x; 1.2115x over previous
"""Multi-head attention (B=2, N=2048, C=768, H=12) on 8 TRN2 NeuronCores.

Sharding: core c = 4*b + g handles batch b (data parallel) and heads
3g..3g+2 (tensor parallel on H). Each core computes its 3 heads end-to-end
plus the partial projection with its 192 rows of w_proj; the host sums the
4 partials per batch and adds b_proj. No cross-device communication.

Clock strategy: the PE reaches/holds 2.4 GHz only while its instruction
stream never waits on a semaphore; switching the PE tiling mode costs
~106ns per transition. Steady state is grouped so the PE always outpaces
ACT (the exp engine, ~1.04us per [128,1024] tile = the true floor) and
therefore never stalls:

  per 2-m-chunk group:  [scores m, m+1]   64-row-tile T0/T8 pairs (~432ns)
                        [attn@v m-3, m-2] full-K matmuls         (~930ns)
                        [one filler item]  full-K qk/v/proj piece (~300-600ns)
ACT does exp only; DVE does all PSUM evacuation + softmax normalization;
junk matmuls pad slots when no real filler remains.
"""

from collections import deque

import ml_dtypes
import numpy as np

import concourse.bass as bass
import concourse.mybir as mybir
import concourse.tile as tile
from concourse import bacc
from concourse.bass_utils import run_bass_kernel_spmd

F32 = mybir.dt.float32
F32R = mybir.dt.float32r
BF16 = mybir.dt.bfloat16
EXP = mybir.ActivationFunctionType.Exp
MULT = mybir.AluOpType.mult

B, N, C = 2, 2048, 768
H = 12
D = 64
HPC = 3  # heads per core
KC = 6  # contraction chunks of 128 over C
NB = 1024  # n-block for attention stage
MC = N // 128  # 16 m-chunks (context)
SCALE = D ** -0.5
LAG = 2  # attn@v trails exp by this many m-chunks

_NC_CACHE = None


def build_nc():
    nc = bacc.Bacc("TRN2", target_bir_lowering=False, debug=False, num_devices=8)
    xt = nc.declare_dram_parameter("xt", [C, N], BF16, isOutput=False)
    wqk = nc.declare_dram_parameter("wqk", [C, HPC * 128], BF16, isOutput=False)
    wv = nc.declare_dram_parameter("wv", [C, HPC * D], BF16, isOutput=False)
    wp = nc.declare_dram_parameter("wp", [HPC * D, C], BF16, isOutput=False)
    out = nc.declare_dram_parameter("out", [N, C], BF16, isOutput=True)

    with tile.TileContext(nc) as tc:
        with (
            tc.tile_pool(name="sb", bufs=1) as sb,
            tc.tile_pool(name="ps", bufs=1, space="PSUM") as ps,
            tc.tile_pool(name="drp", bufs=2, space="DRAM") as drp,
        ):
            # ---- input loads --------------------------------------------
            xtb = sb.tile([128, KC * N], BF16, tag="xtb")
            wqkb = sb.tile([128, KC * HPC * 128], BF16, tag="wqkb")
            wvb = sb.tile([128, KC * HPC * D], BF16, tag="wvb")
            for kc in range(KC):
                nc.sync.dma_start(
                    wqkb[:, kc * HPC * 128 : (kc + 1) * HPC * 128],
                    wqk[kc * 128 : (kc + 1) * 128, :],
                )
                nc.sync.dma_start(
                    wvb[:, kc * HPC * D : (kc + 1) * HPC * D],
                    wv[kc * 128 : (kc + 1) * 128, :],
                )
                nc.sync.dma_start(
                    xtb[:, kc * N : (kc + 1) * N], xt[kc * 128 : (kc + 1) * 128, :]
                )
            wp01 = sb.tile([128, C], BF16, tag="wp01")
            nc.sync.dma_start(wp01[:], wp[0:128, :])
            wp2 = sb.tile([64, C], BF16, tag="wp2")
            nc.sync.dma_start(wp2[:], wp[128 : HPC * D, :])

            junk = sb.tile([128, 512], BF16, tag="junk")
            nc.vector.memset(junk[:], 1.0)

            ones_f = sb.tile([128, MC], F32, tag="ones_f")
            nc.vector.memset(ones_f[:], 1.0)
            v_sb = sb.tile([128, HPC * MC * 65], F32R, tag="v")
            v4 = v_sb.rearrange("p (h m w) -> p h m w", h=HPC, m=MC)
            for h in range(HPC):
                nc.vector.tensor_copy(v4[:, h, :, 64], ones_f[:, :])

            qk_sb = [
                sb.tile([128, N], BF16, tag=f"qk{h}", name=f"qk{h}") for h in range(HPC)
            ]
            kq_sb = [
                sb.tile([128, N], BF16, tag=f"kq{h}", name=f"kq{h}") for h in range(HPC)
            ]
            stk = sb.tile([128, N], BF16, tag="stk")
            outT1 = sb.tile([64, N], BF16, tag="outT1")
            outT2 = sb.tile([64, N], BF16, tag="outT2")

            def sc_tile(name):
                return ps.tile([128, NB], F32, tag="sc", bufs=3, name=name)

            # ---- lead-in (full mode): ramp + qk head 0 ------------------
            for i in range(14):
                psw = sc_tile(f"junk{i}")
                nc.tensor.matmul(
                    psw[:, 0:512], junk[:, 0:128], junk[:], start=True, stop=True
                )
                nc.tensor.matmul(
                    psw[:, 512:1024], junk[:, 0:128], junk[:], start=True, stop=True
                )

            def emit_qk_half(h, half, psq, kc):
                hb = half * 1024
                for s in range(2):
                    nc.tensor.matmul(
                        psq[:, s * 512 : (s + 1) * 512],
                        wqkb[
                            :, kc * HPC * 128 + h * 128 : kc * HPC * 128 + (h + 1) * 128
                        ],
                        xtb[:, kc * N + hb + s * 512 : kc * N + hb + (s + 1) * 512],
                        start=(kc == 0),
                        stop=(kc == KC - 1),
                    )

            def emit_qk_tail(h, half, psq):
                hb = half * 1024
                nc.vector.tensor_copy(qk_sb[h][:, hb : hb + 1024], psq[:])
                nc.sync.dma_start(
                    kq_sb[h][0:64, hb : hb + 1024], qk_sb[h][64:128, hb : hb + 1024]
                )
                nc.sync.dma_start(
                    kq_sb[h][64:128, hb : hb + 1024], qk_sb[h][0:64, hb : hb + 1024]
                )

            for half in range(2):
                psq = sc_tile(f"qk0h{half}")
                for kc in range(KC):
                    emit_qk_half(0, half, psq, kc)
                emit_qk_tail(0, half, psq)

            # ---- background PE work (full mode) -------------------------
            bg = deque()

            def v_item(m):
                def emit():
                    psv = sc_tile(f"v{m}")
                    for kc in range(KC):
                        nc.tensor.matmul(
                            psv[:, 0 : HPC * D],
                            xtb[:, kc * N + m * 128 : kc * N + (m + 1) * 128],
                            wvb[:, kc * HPC * D : (kc + 1) * HPC * D],
                            start=(kc == 0),
                            stop=(kc == KC - 1),
                        )
                    nc.vector.tensor_copy(
                        v4[:, :, m, 0:64],
                        psv[:, 0 : HPC * D].rearrange("p (h d) -> p h d", h=HPC),
                    )

                return emit

            def qk_item(h, half, state, step):
                # step 0..KC-1: one kc contraction step (2 matmuls ~432ns);
                # step KC: evacuation copy + partition swap (no PE work)
                def emit():
                    if step == 0:
                        state["ps"] = sc_tile(f"qk{h}h{half}")
                    if step < KC:
                        emit_qk_half(h, half, state["ps"], step)
                    else:
                        emit_qk_tail(h, half, state["ps"])

                return emit

            def proj_item(k):
                def emit():
                    pp = sc_tile(f"pp{k}")
                    for sw, w in ((0, 512), (512, 256)):
                        nc.tensor.matmul(
                            pp[:, sw : sw + w],
                            stk[:, k * 128 : (k + 1) * 128],
                            wp01[:, sw : sw + w],
                            start=True,
                            stop=False,
                        )
                    for sw, w in ((0, 512), (512, 256)):
                        nc.tensor.matmul(
                            pp[:, sw : sw + w],
                            outT2[0:64, k * 128 : (k + 1) * 128],
                            wp2[:, sw : sw + w],
                            start=False,
                            stop=True,
                        )
                    ob = sb.tile([128, C], BF16, tag="ob", bufs=3, name=f"ob{k}")
                    nc.vector.tensor_copy(ob[:], pp[:, 0:C])
                    nc.sync.dma_start(out[k * 128 : (k + 1) * 128, :], ob[:])

                return emit

            for m in range(MC):
                bg.append(v_item(m))
            for h in (1, 2):
                for half in range(2):
                    state = {}
                    for step in range(KC + 1):
                        bg.append(qk_item(h, half, state, step))

            pad_idx = [0]

            def pump(n, pad=False):
                for _ in range(n):
                    if bg:
                        bg.popleft()()
                    elif pad:
                        psw = sc_tile(f"pad{pad_idx[0]}")
                        pad_idx[0] += 1
                        for s in range(2):
                            nc.tensor.matmul(
                                psw[:, s * 512 : (s + 1) * 512],
                                junk[:, 0:128],
                                junk[:],
                                start=True,
                                stop=True,
                            )

            # ---- attention: head outer, nb inner ------------------------
            for h in range(HPC):
                for nb in range(N // NB):
                    ex_tiles = {}
                    oa = None

                    def emit_scores(m):
                        sc = sc_tile(f"sc{h}_{nb}_{m}")
                        nc.tensor.matmul(
                            sc[:, 0:512],
                            kq_sb[h][0:64, m * 128 : (m + 1) * 128],
                            qk_sb[h][0:64, nb * NB : nb * NB + 512],
                            start=True,
                            stop=True,
                            tile_position=(0, 0),
                        )
                        nc.tensor.matmul(
                            sc[:, 512:1024],
                            qk_sb[h][64:128, m * 128 : (m + 1) * 128],
                            kq_sb[h][64:128, nb * NB + 512 : nb * NB + 1024],
                            start=True,
                            stop=True,
                            tile_position=(64, 0),
                        )
                        ex = sb.tile([128, NB], F32R, tag="ex", bufs=6)
                        nc.scalar.activation(ex[:], sc[:], EXP, scale=SCALE)
                        ex_tiles[m] = ex

                    def emit_oa(m):
                        exm = ex_tiles.pop(m)
                        for s in range(2):
                            nc.tensor.matmul(
                                oa[:, s * 512 : (s + 1) * 512],
                                v4[:, h, m, :],
                                exm[:, s * 512 : (s + 1) * 512],
                                start=(m == 0),
                                stop=(m == MC - 1),
                            )

                    for g in range(MC // 2):
                        m = 2 * g
                        emit_scores(m)
                        emit_scores(m + 1)
                        if m == LAG:
                            oa = ps.tile([65, NB], F32, tag="oa", bufs=1)
                        if m >= LAG:
                            emit_oa(m - LAG)
                            emit_oa(m - LAG + 1)
                        pump(2, pad=True)
                    for m in range(MC - LAG, MC):
                        emit_oa(m)
                    pump(1)

                    # epilogue: softmax normalization (no PE involvement)
                    cs = sb.tile([1, NB], F32, tag="cs", bufs=2)
                    nc.vector.tensor_copy(cs[:], oa[64:65, :])
                    osb = sb.tile([64, NB], F32, tag="osb", bufs=3)
                    nc.vector.tensor_copy(osb[:], oa[0:64, :])
                    rf = sb.tile([1, NB], F32, tag="rf", bufs=2)
                    nc.vector.reciprocal_approx_fast(out=rf[:], in_=cs[:])
                    rfd = drp.tile([1, NB], F32, tag="rfd", bufs=2)
                    nc.sync.dma_start(rfd[:], rf[:])
                    rbs = sb.tile([64, NB], F32, tag="rbs", bufs=2)
                    nc.sync.dma_start(rbs[:], rfd[:].partition_broadcast(64))
                    if h == 0:
                        mdst = stk[0:64, nb * NB : (nb + 1) * NB]
                    elif h == 1:
                        mdst = outT1[0:64, nb * NB : (nb + 1) * NB]
                    else:
                        mdst = outT2[0:64, nb * NB : (nb + 1) * NB]
                    nc.vector.tensor_tensor(out=mdst, in0=osb[:], in1=rbs[:], op=MULT)
                    if h == 1:
                        nc.sync.dma_start(
                            stk[64:128, nb * NB : (nb + 1) * NB],
                            outT1[0:64, nb * NB : (nb + 1) * NB],
                        )
                    if h == HPC - 1:
                        for k in range(nb * 8, nb * 8 + 8):
                            bg.append(proj_item(k))

            pump(len(bg))

    nc.compile()
    return nc


def get_nc():
    global _NC_CACHE
    if _NC_CACHE is None:
        _NC_CACHE = build_nc()
    return _NC_CACHE


def make_in_maps(x, w_qkv, w_proj):
    """Shard inputs for the 8 cores: core c = 4*b + g."""
    in_maps = []
    for c in range(8):
        b, g = divmod(c, 4)
        heads = [3 * g + h for h in range(HPC)]
        xt = np.ascontiguousarray(x[b].T).astype(ml_dtypes.bfloat16)
        wqk = np.empty((C, HPC * 128), dtype=ml_dtypes.bfloat16)
        wv = np.empty((C, HPC * D), dtype=ml_dtypes.bfloat16)
        for i, hh in enumerate(heads):
            wqk[:, i * 128 : i * 128 + 64] = w_qkv[:, hh * D : (hh + 1) * D]
            wqk[:, i * 128 + 64 : i * 128 + 128] = w_qkv[
                :, C + hh * D : C + (hh + 1) * D
            ]
            wv[:, i * D : (i + 1) * D] = w_qkv[:, 2 * C + hh * D : 2 * C + (hh + 1) * D]
        wp = np.ascontiguousarray(w_proj[g * HPC * D : (g + 1) * HPC * D, :]).astype(
            ml_dtypes.bfloat16
        )
        in_maps.append(
            {"xt": xt, "wqk": np.ascontiguousarray(wqk), "wv": wv, "wp": wp}
        )
    return in_maps


def run(x, w_qkv, w_proj, b_proj, trace=False):
    nc = get_nc()
    in_maps = make_in_maps(x, w_qkv, w_proj)
    res = run_bass_kernel_spmd(nc, in_maps, core_ids=list(range(8)), trace=trace)
    out = np.empty((B, N, C), dtype=np.float32)
    for b in range(B):
        acc = res.results[4 * b]["out"].astype(np.float32)
        for g in range(1, 4):
            acc = acc + res.results[4 * b + g]["out"]
        out[b] = acc + b_proj[None, :].astype(np.float32)
    return out, res


def kernel(x, w_qkv, w_proj, b_proj):
    out, _ = run(
        np.asarray(x), np.asarray(w_qkv), np.asarray(w_proj), np.asarray(b_proj)
    )
    return out


# revision 19
# speedup vs baseline: 1.2181x; 1.0054x over previous
"""Multi-head attention (B=2, N=2048, C=768, H=12) on 8 TRN2 NeuronCores.

Sharding: core c = 4*b + g handles batch b (data parallel) and heads
3g..3g+2 (tensor parallel on H). Each core computes its 3 heads end-to-end
plus the partial projection with its 192 rows of w_proj; the host sums the
4 partials per batch and adds b_proj. No cross-device communication.

Clock strategy: the PE reaches/holds 2.4 GHz only while its instruction
stream never waits on a semaphore; switching the PE tiling mode costs
~106ns per transition. Steady state is grouped so the PE always outpaces
ACT (the exp engine, ~1.04us per [128,1024] tile = the true floor) and
therefore never stalls:

  per 2-m-chunk group:  [scores m, m+1]   64-row-tile T0/T8 pairs (~432ns)
                        [attn@v m-3, m-2] full-K matmuls         (~930ns)
                        [one filler item]  full-K qk/v/proj piece (~300-600ns)
ACT does exp only; DVE does all PSUM evacuation + softmax normalization;
junk matmuls pad slots when no real filler remains.
"""

from collections import deque

import ml_dtypes
import numpy as np

import concourse.bass as bass
import concourse.mybir as mybir
import concourse.tile as tile
from concourse import bacc
from concourse.bass_utils import run_bass_kernel_spmd

F32 = mybir.dt.float32
F32R = mybir.dt.float32r
BF16 = mybir.dt.bfloat16
EXP = mybir.ActivationFunctionType.Exp
MULT = mybir.AluOpType.mult

B, N, C = 2, 2048, 768
H = 12
D = 64
HPC = 3  # heads per core
KC = 6  # contraction chunks of 128 over C
NB = 1024  # n-block for attention stage
MC = N // 128  # 16 m-chunks (context)
SCALE = D ** -0.5
LAG = 2  # attn@v trails exp by this many m-chunks

_NC_CACHE = None


def build_nc():
    nc = bacc.Bacc("TRN2", target_bir_lowering=False, debug=False, num_devices=8)
    xt = nc.declare_dram_parameter("xt", [C, N], BF16, isOutput=False)
    wqk = nc.declare_dram_parameter("wqk", [C, HPC * 128], BF16, isOutput=False)
    wv = nc.declare_dram_parameter("wv", [C, HPC * D], BF16, isOutput=False)
    wp = nc.declare_dram_parameter("wp", [HPC * D, C], BF16, isOutput=False)
    out = nc.declare_dram_parameter("out", [N, C], BF16, isOutput=True)

    with tile.TileContext(nc) as tc:
        with (
            tc.tile_pool(name="sb", bufs=1) as sb,
            tc.tile_pool(name="ps", bufs=1, space="PSUM") as ps,
            tc.tile_pool(name="drp", bufs=2, space="DRAM") as drp,
        ):
            # ---- input loads --------------------------------------------
            xtb = sb.tile([128, KC * N], BF16, tag="xtb")
            wqkb = sb.tile([128, KC * HPC * 128], BF16, tag="wqkb")
            wvb = sb.tile([128, KC * HPC * D], BF16, tag="wvb")
            for kc in range(KC):
                nc.sync.dma_start(
                    wqkb[:, kc * HPC * 128 : (kc + 1) * HPC * 128],
                    wqk[kc * 128 : (kc + 1) * 128, :],
                )
                nc.sync.dma_start(
                    wvb[:, kc * HPC * D : (kc + 1) * HPC * D],
                    wv[kc * 128 : (kc + 1) * 128, :],
                )
                nc.sync.dma_start(
                    xtb[:, kc * N : (kc + 1) * N], xt[kc * 128 : (kc + 1) * 128, :]
                )
            wp01 = sb.tile([128, C], BF16, tag="wp01")
            nc.sync.dma_start(wp01[:], wp[0:128, :])
            wp2 = sb.tile([64, C], BF16, tag="wp2")
            nc.sync.dma_start(wp2[:], wp[128 : HPC * D, :])

            junk = sb.tile([128, 512], BF16, tag="junk")
            nc.vector.memset(junk[:], 1.0)

            ones_f = sb.tile([128, MC], F32, tag="ones_f")
            nc.vector.memset(ones_f[:], 1.0)
            v_sb = sb.tile([128, HPC * MC * 65], F32R, tag="v")
            v4 = v_sb.rearrange("p (h m w) -> p h m w", h=HPC, m=MC)
            for h in range(HPC):
                nc.vector.tensor_copy(v4[:, h, :, 64], ones_f[:, :])

            qk_sb = [
                sb.tile([128, N], BF16, tag=f"qk{h}", name=f"qk{h}") for h in range(HPC)
            ]
            kq_sb = [
                sb.tile([128, N], BF16, tag=f"kq{h}", name=f"kq{h}") for h in range(HPC)
            ]
            stk = sb.tile([128, N], BF16, tag="stk")
            outT1 = sb.tile([64, N], BF16, tag="outT1")
            outT2 = sb.tile([64, N], BF16, tag="outT2")

            def sc_tile(name):
                return ps.tile([128, NB], F32, tag="sc", bufs=3, name=name)

            # ---- lead-in (full mode): ramp + qk head 0 ------------------
            for i in range(4):
                psw = sc_tile(f"junk{i}")
                nc.tensor.matmul(
                    psw[:, 0:512], junk[:, 0:128], junk[:], start=True, stop=True
                )
                nc.tensor.matmul(
                    psw[:, 512:1024], junk[:, 0:128], junk[:], start=True, stop=True
                )

            def emit_qk_half(h, half, psq, kc):
                hb = half * 1024
                for s in range(2):
                    nc.tensor.matmul(
                        psq[:, s * 512 : (s + 1) * 512],
                        wqkb[
                            :, kc * HPC * 128 + h * 128 : kc * HPC * 128 + (h + 1) * 128
                        ],
                        xtb[:, kc * N + hb + s * 512 : kc * N + hb + (s + 1) * 512],
                        start=(kc == 0),
                        stop=(kc == KC - 1),
                    )

            def emit_qk_tail(h, half, psq):
                hb = half * 1024
                nc.vector.tensor_copy(qk_sb[h][:, hb : hb + 1024], psq[:])
                nc.sync.dma_start(
                    kq_sb[h][0:64, hb : hb + 1024], qk_sb[h][64:128, hb : hb + 1024]
                )
                nc.sync.dma_start(
                    kq_sb[h][64:128, hb : hb + 1024], qk_sb[h][0:64, hb : hb + 1024]
                )

            for half in range(2):
                psq = sc_tile(f"qk0h{half}")
                for kc in range(KC):
                    emit_qk_half(0, half, psq, kc)
                emit_qk_tail(0, half, psq)

            # ---- background PE work (full mode) -------------------------
            bg = deque()

            def v_item(m):
                def emit():
                    psv = sc_tile(f"v{m}")
                    for kc in range(KC):
                        nc.tensor.matmul(
                            psv[:, 0 : HPC * D],
                            xtb[:, kc * N + m * 128 : kc * N + (m + 1) * 128],
                            wvb[:, kc * HPC * D : (kc + 1) * HPC * D],
                            start=(kc == 0),
                            stop=(kc == KC - 1),
                        )
                    nc.vector.tensor_copy(
                        v4[:, :, m, 0:64],
                        psv[:, 0 : HPC * D].rearrange("p (h d) -> p h d", h=HPC),
                    )

                return emit

            def qk_item(h, half, state, step):
                # step 0..KC-1: one kc contraction step (2 matmuls ~432ns);
                # step KC: evacuation copy + partition swap (no PE work)
                def emit():
                    if step == 0:
                        state["ps"] = sc_tile(f"qk{h}h{half}")
                    if step < KC:
                        emit_qk_half(h, half, state["ps"], step)
                    else:
                        emit_qk_tail(h, half, state["ps"])

                return emit

            def proj_item(k):
                def emit():
                    pp = sc_tile(f"pp{k}")
                    for sw, w in ((0, 512), (512, 256)):
                        nc.tensor.matmul(
                            pp[:, sw : sw + w],
                            stk[:, k * 128 : (k + 1) * 128],
                            wp01[:, sw : sw + w],
                            start=True,
                            stop=False,
                        )
                    for sw, w in ((0, 512), (512, 256)):
                        nc.tensor.matmul(
                            pp[:, sw : sw + w],
                            outT2[0:64, k * 128 : (k + 1) * 128],
                            wp2[:, sw : sw + w],
                            start=False,
                            stop=True,
                        )
                    ob = sb.tile([128, C], BF16, tag="ob", bufs=3, name=f"ob{k}")
                    nc.vector.tensor_copy(ob[:], pp[:, 0:C])
                    nc.sync.dma_start(out[k * 128 : (k + 1) * 128, :], ob[:])

                return emit

            for m in range(MC):
                bg.append((640, v_item(m)))
            for h in (1, 2):
                for half in range(2):
                    state = {}
                    for step in range(KC + 1):
                        bg.append((440 if step < KC else 0, qk_item(h, half, state, step)))

            pad_idx = [0]

            def emit_pad(n_mm):
                psw = sc_tile(f"pad{pad_idx[0]}")
                pad_idx[0] += 1
                for s in range(n_mm):
                    nc.tensor.matmul(
                        psw[:, (s % 2) * 512 : (s % 2 + 1) * 512],
                        junk[:, 0:128],
                        junk[:],
                        start=True,
                        stop=True,
                    )

            def pump(budget, pad=False):
                # consume bg items until the PE-time budget is spent; if the
                # queue is dry, emit ONE junk pad tile to fill the remainder
                while bg and budget > 0:
                    cost, emit = bg.popleft()
                    emit()
                    budget -= cost
                if pad and budget > 200:
                    emit_pad(min(4, max(2, round(budget / 233))))

            # ---- attention: head outer, nb inner ------------------------
            for h in range(HPC):
                for nb in range(N // NB):
                    ex_tiles = {}
                    oa = None

                    def emit_scores(m):
                        sc = sc_tile(f"sc{h}_{nb}_{m}")
                        nc.tensor.matmul(
                            sc[:, 0:512],
                            kq_sb[h][0:64, m * 128 : (m + 1) * 128],
                            qk_sb[h][0:64, nb * NB : nb * NB + 512],
                            start=True,
                            stop=True,
                            tile_position=(0, 0),
                        )
                        nc.tensor.matmul(
                            sc[:, 512:1024],
                            qk_sb[h][64:128, m * 128 : (m + 1) * 128],
                            kq_sb[h][64:128, nb * NB + 512 : nb * NB + 1024],
                            start=True,
                            stop=True,
                            tile_position=(64, 0),
                        )
                        ex = sb.tile([128, NB], F32R, tag="ex", bufs=6)
                        nc.scalar.activation(ex[:], sc[:], EXP, scale=SCALE)
                        ex_tiles[m] = ex

                    def emit_oa(m):
                        exm = ex_tiles.pop(m)
                        for s in range(2):
                            nc.tensor.matmul(
                                oa[:, s * 512 : (s + 1) * 512],
                                v4[:, h, m, :],
                                exm[:, s * 512 : (s + 1) * 512],
                                start=(m == 0),
                                stop=(m == MC - 1),
                            )

                    for g in range(MC // 2):
                        m = 2 * g
                        emit_scores(m)
                        emit_scores(m + 1)
                        if m == LAG:
                            oa = ps.tile([65, NB], F32, tag="oa", bufs=1)
                        if m >= LAG:
                            emit_oa(m - LAG)
                            emit_oa(m - LAG + 1)
                            pump(950, pad=True)
                        else:
                            pump(1900, pad=True)
                    for m in range(MC - LAG, MC):
                        emit_oa(m)
                    pump(400)

                    # epilogue: softmax normalization (no PE involvement)
                    cs = sb.tile([1, NB], F32, tag="cs", bufs=2)
                    nc.vector.tensor_copy(cs[:], oa[64:65, :])
                    osb = sb.tile([64, NB], F32, tag="osb", bufs=3)
                    nc.vector.tensor_copy(osb[:], oa[0:64, :])
                    rf = sb.tile([1, NB], F32, tag="rf", bufs=2)
                    nc.vector.reciprocal_approx_fast(out=rf[:], in_=cs[:])
                    rfd = drp.tile([1, NB], F32, tag="rfd", bufs=2)
                    nc.sync.dma_start(rfd[:], rf[:])
                    rbs = sb.tile([64, NB], F32, tag="rbs", bufs=2)
                    nc.sync.dma_start(rbs[:], rfd[:].partition_broadcast(64))
                    if h == 0:
                        mdst = stk[0:64, nb * NB : (nb + 1) * NB]
                    elif h == 1:
                        mdst = outT1[0:64, nb * NB : (nb + 1) * NB]
                    else:
                        mdst = outT2[0:64, nb * NB : (nb + 1) * NB]
                    nc.vector.tensor_tensor(out=mdst, in0=osb[:], in1=rbs[:], op=MULT)
                    if h == 1:
                        nc.sync.dma_start(
                            stk[64:128, nb * NB : (nb + 1) * NB],
                            outT1[0:64, nb * NB : (nb + 1) * NB],
                        )
                    if h == HPC - 1:
                        for k in range(nb * 8, nb * 8 + 8):
                            bg.append((960, proj_item(k)))

            for _ in range(5):
                emit_pad(4)
            pump(10**9)

    nc.compile()
    return nc


def get_nc():
    global _NC_CACHE
    if _NC_CACHE is None:
        _NC_CACHE = build_nc()
    return _NC_CACHE


def make_in_maps(x, w_qkv, w_proj):
    """Shard inputs for the 8 cores: core c = 4*b + g."""
    in_maps = []
    for c in range(8):
        b, g = divmod(c, 4)
        heads = [3 * g + h for h in range(HPC)]
        xt = np.ascontiguousarray(x[b].T).astype(ml_dtypes.bfloat16)
        wqk = np.empty((C, HPC * 128), dtype=ml_dtypes.bfloat16)
        wv = np.empty((C, HPC * D), dtype=ml_dtypes.bfloat16)
        for i, hh in enumerate(heads):
            wqk[:, i * 128 : i * 128 + 64] = w_qkv[:, hh * D : (hh + 1) * D]
            wqk[:, i * 128 + 64 : i * 128 + 128] = w_qkv[
                :, C + hh * D : C + (hh + 1) * D
            ]
            wv[:, i * D : (i + 1) * D] = w_qkv[:, 2 * C + hh * D : 2 * C + (hh + 1) * D]
        wp = np.ascontiguousarray(w_proj[g * HPC * D : (g + 1) * HPC * D, :]).astype(
            ml_dtypes.bfloat16
        )
        in_maps.append(
            {"xt": xt, "wqk": np.ascontiguousarray(wqk), "wv": wv, "wp": wp}
        )
    return in_maps


def run(x, w_qkv, w_proj, b_proj, trace=False):
    nc = get_nc()
    in_maps = make_in_maps(x, w_qkv, w_proj)
    res = run_bass_kernel_spmd(nc, in_maps, core_ids=list(range(8)), trace=trace)
    out = np.empty((B, N, C), dtype=np.float32)
    for b in range(B):
        acc = res.results[4 * b]["out"].astype(np.float32)
        for g in range(1, 4):
            acc = acc + res.results[4 * b + g]["out"]
        out[b] = acc + b_proj[None, :].astype(np.float32)
    return out, res


def kernel(x, w_qkv, w_proj, b_proj):
    out, _ = run(
        np.asarray(x), np.asarray(w_qkv), np.asarray(w_proj), np.asarray(b_proj)
    )
    return out


# revision 21
# speedup vs baseline: 1.2349x; 1.0138x over previous
"""Multi-head attention (B=2, N=2048, C=768, H=12) on 8 TRN2 NeuronCores.

Sharding: core c = 4*b + g handles batch b (data parallel) and heads
3g..3g+2 (tensor parallel on H). Each core computes its 3 heads end-to-end
plus the partial projection with its 192 rows of w_proj; the host sums the
4 partials per batch and adds b_proj. No cross-device communication.

Clock strategy: the PE reaches/holds 2.4 GHz only while its instruction
stream never waits on a semaphore; switching the PE tiling mode costs
~106ns per transition. Steady state is grouped so the PE always outpaces
ACT (the exp engine, ~1.04us per [128,1024] tile = the true floor) and
therefore never stalls:

  per 2-m-chunk group:  [scores m, m+1]   64-row-tile T0/T8 pairs (~432ns)
                        [attn@v m-3, m-2] full-K matmuls         (~930ns)
                        [one filler item]  full-K qk/v/proj piece (~300-600ns)
ACT does exp only; DVE does all PSUM evacuation + softmax normalization;
junk matmuls pad slots when no real filler remains.
"""

from collections import deque

import ml_dtypes
import numpy as np

import concourse.bass as bass
import concourse.mybir as mybir
import concourse.tile as tile
from concourse import bacc
from concourse.bass_utils import run_bass_kernel_spmd

F32 = mybir.dt.float32
F32R = mybir.dt.float32r
BF16 = mybir.dt.bfloat16
EXP = mybir.ActivationFunctionType.Exp
MULT = mybir.AluOpType.mult

B, N, C = 2, 2048, 768
H = 12
D = 64
HPC = 3  # heads per core
KC = 6  # contraction chunks of 128 over C
NB = 1024  # n-block for attention stage
MC = N // 128  # 16 m-chunks (context)
SCALE = D ** -0.5
LAG = 4  # attn@v trails exp by this many m-chunks

_NC_CACHE = None


def build_nc():
    nc = bacc.Bacc("TRN2", target_bir_lowering=False, debug=False, num_devices=8)
    xt = nc.declare_dram_parameter("xt", [C, N], BF16, isOutput=False)
    wqk = nc.declare_dram_parameter("wqk", [C, HPC * 128], BF16, isOutput=False)
    wv = nc.declare_dram_parameter("wv", [C, HPC * D], BF16, isOutput=False)
    wp = nc.declare_dram_parameter("wp", [HPC * D, C], BF16, isOutput=False)
    out = nc.declare_dram_parameter("out", [N, C], BF16, isOutput=True)

    with tile.TileContext(nc) as tc:
        with (
            tc.tile_pool(name="sb", bufs=1) as sb,
            tc.tile_pool(name="ps", bufs=1, space="PSUM") as ps,
            tc.tile_pool(name="drp", bufs=2, space="DRAM") as drp,
        ):
            # ---- input loads --------------------------------------------
            xtb = sb.tile([128, KC * N], BF16, tag="xtb")
            wqkb = sb.tile([128, KC * HPC * 128], BF16, tag="wqkb")
            wvb = sb.tile([128, KC * HPC * D], BF16, tag="wvb")
            for kc in range(KC):
                nc.sync.dma_start(
                    wqkb[:, kc * HPC * 128 : (kc + 1) * HPC * 128],
                    wqk[kc * 128 : (kc + 1) * 128, :],
                )
                nc.sync.dma_start(
                    wvb[:, kc * HPC * D : (kc + 1) * HPC * D],
                    wv[kc * 128 : (kc + 1) * 128, :],
                )
            for sl in range(4):
                for kc in range(KC):
                    nc.sync.dma_start(
                        xtb[:, kc * N + sl * 512 : kc * N + (sl + 1) * 512],
                        xt[kc * 128 : (kc + 1) * 128, sl * 512 : (sl + 1) * 512],
                    )
            wp01 = sb.tile([128, C], BF16, tag="wp01")
            nc.sync.dma_start(wp01[:], wp[0:128, :])
            wp2 = sb.tile([64, C], BF16, tag="wp2")
            nc.sync.dma_start(wp2[:], wp[128 : HPC * D, :])

            junk = sb.tile([128, 512], BF16, tag="junk")
            nc.vector.memset(junk[:], 1.0)

            ones_f = sb.tile([128, MC], F32, tag="ones_f")
            nc.vector.memset(ones_f[:], 1.0)
            v_sb = sb.tile([128, HPC * MC * 65], F32R, tag="v")
            v4 = v_sb.rearrange("p (h m w) -> p h m w", h=HPC, m=MC)
            for h in range(HPC):
                nc.vector.tensor_copy(v4[:, h, :, 64], ones_f[:, :])

            qk_sb = [
                sb.tile([128, N], BF16, tag=f"qk{h}", name=f"qk{h}") for h in range(HPC)
            ]
            kq_sb = [
                sb.tile([128, N], BF16, tag=f"kq{h}", name=f"kq{h}") for h in range(HPC)
            ]
            stk = sb.tile([128, N], BF16, tag="stk")
            outT1 = sb.tile([64, N], BF16, tag="outT1")
            outT2 = sb.tile([64, N], BF16, tag="outT2")

            def sc_tile(name):
                return ps.tile([128, NB], F32, tag="sc", bufs=3, name=name)

            # ---- lead-in (full mode): ramp + qk head 0 ------------------
            for i in range(4):
                psw = sc_tile(f"junk{i}")
                nc.tensor.matmul(
                    psw[:, 0:512], junk[:, 0:128], junk[:], start=True, stop=True
                )
                nc.tensor.matmul(
                    psw[:, 512:1024], junk[:, 0:128], junk[:], start=True, stop=True
                )

            def emit_qk_half(h, half, psq, kc):
                hb = half * 1024
                for s in range(2):
                    nc.tensor.matmul(
                        psq[:, s * 512 : (s + 1) * 512],
                        wqkb[
                            :, kc * HPC * 128 + h * 128 : kc * HPC * 128 + (h + 1) * 128
                        ],
                        xtb[:, kc * N + hb + s * 512 : kc * N + hb + (s + 1) * 512],
                        start=(kc == 0),
                        stop=(kc == KC - 1),
                    )

            def emit_qk_tail(h, half, psq):
                hb = half * 1024
                nc.vector.tensor_copy(qk_sb[h][:, hb : hb + 1024], psq[:])
                nc.sync.dma_start(
                    kq_sb[h][0:64, hb : hb + 1024], qk_sb[h][64:128, hb : hb + 1024]
                )
                nc.sync.dma_start(
                    kq_sb[h][64:128, hb : hb + 1024], qk_sb[h][0:64, hb : hb + 1024]
                )

            for half in range(2):
                psq = sc_tile(f"qk0h{half}")
                for kc in range(KC):
                    emit_qk_half(0, half, psq, kc)
                emit_qk_tail(0, half, psq)

            # ---- background PE work (full mode) -------------------------
            bg = deque()

            def v_item(m):
                def emit():
                    psv = sc_tile(f"v{m}")
                    for kc in range(KC):
                        nc.tensor.matmul(
                            psv[:, 0 : HPC * D],
                            xtb[:, kc * N + m * 128 : kc * N + (m + 1) * 128],
                            wvb[:, kc * HPC * D : (kc + 1) * HPC * D],
                            start=(kc == 0),
                            stop=(kc == KC - 1),
                        )
                    nc.vector.tensor_copy(
                        v4[:, :, m, 0:64],
                        psv[:, 0 : HPC * D].rearrange("p (h d) -> p h d", h=HPC),
                    )

                return emit

            def qk_item(h, half, state, step):
                # step 0..KC-1: one kc contraction step (2 matmuls ~432ns);
                # step KC: evacuation copy + partition swap (no PE work)
                def emit():
                    if step == 0:
                        state["ps"] = sc_tile(f"qk{h}h{half}")
                    if step < KC:
                        emit_qk_half(h, half, state["ps"], step)
                    else:
                        emit_qk_tail(h, half, state["ps"])

                return emit

            def proj_item(k):
                def emit():
                    pp = sc_tile(f"pp{k}")
                    for sw, w in ((0, 512), (512, 256)):
                        nc.tensor.matmul(
                            pp[:, sw : sw + w],
                            stk[:, k * 128 : (k + 1) * 128],
                            wp01[:, sw : sw + w],
                            start=True,
                            stop=False,
                        )
                    for sw, w in ((0, 512), (512, 256)):
                        nc.tensor.matmul(
                            pp[:, sw : sw + w],
                            outT2[0:64, k * 128 : (k + 1) * 128],
                            wp2[:, sw : sw + w],
                            start=False,
                            stop=True,
                        )
                    ob = sb.tile([128, C], BF16, tag="ob", bufs=3, name=f"ob{k}")
                    nc.vector.tensor_copy(ob[:], pp[:, 0:C])
                    nc.sync.dma_start(out[k * 128 : (k + 1) * 128, :], ob[:])

                return emit

            for m in range(MC):
                bg.append((640, v_item(m), False))
            for h in (1, 2):
                for half in range(2):
                    state = {}
                    for step in range(KC + 1):
                        bg.append(
                            (440 if step < KC else 0, qk_item(h, half, state, step), False)
                        )

            pad_idx = [0]

            def emit_pad(n_mm):
                psw = sc_tile(f"pad{pad_idx[0]}")
                pad_idx[0] += 1
                for s in range(n_mm):
                    nc.tensor.matmul(
                        psw[:, (s % 2) * 512 : (s % 2 + 1) * 512],
                        junk[:, 0:128],
                        junk[:],
                        start=True,
                        stop=True,
                    )

            def pump(budget, pad=False, allow_proj=True):
                # consume bg items until the PE-time budget is spent; if the
                # queue is dry, emit ONE junk pad tile to fill the remainder
                while bg and budget > 0:
                    if bg[0][2] and not allow_proj:
                        break
                    cost, emit, _ = bg.popleft()
                    emit()
                    budget -= cost
                if pad and budget > 200:
                    emit_pad(min(4, max(2, round(budget / 233))))

            # ---- attention: head outer, nb inner ------------------------
            for h in range(HPC):
                for nb in range(N // NB):
                    ex_tiles = {}
                    oa = None

                    def emit_scores(m):
                        sc = sc_tile(f"sc{h}_{nb}_{m}")
                        nc.tensor.matmul(
                            sc[:, 0:512],
                            kq_sb[h][0:64, m * 128 : (m + 1) * 128],
                            qk_sb[h][0:64, nb * NB : nb * NB + 512],
                            start=True,
                            stop=True,
                            tile_position=(0, 0),
                        )
                        nc.tensor.matmul(
                            sc[:, 512:1024],
                            qk_sb[h][64:128, m * 128 : (m + 1) * 128],
                            kq_sb[h][64:128, nb * NB + 512 : nb * NB + 1024],
                            start=True,
                            stop=True,
                            tile_position=(64, 0),
                        )
                        ex = sb.tile([128, NB], F32R, tag="ex", bufs=7)
                        nc.scalar.activation(ex[:], sc[:], EXP, scale=SCALE)
                        ex_tiles[m] = ex

                    def emit_oa(m):
                        exm = ex_tiles.pop(m)
                        for s in range(2):
                            nc.tensor.matmul(
                                oa[:, s * 512 : (s + 1) * 512],
                                v4[:, h, m, :],
                                exm[:, s * 512 : (s + 1) * 512],
                                start=(m == 0),
                                stop=(m == MC - 1),
                            )

                    for g in range(MC // 2):
                        m = 2 * g
                        emit_scores(m)
                        emit_scores(m + 1)
                        if m == LAG:
                            oa = ps.tile([65, NB], F32, tag="oa", bufs=1)
                        if m >= LAG:
                            emit_oa(m - LAG)
                            emit_oa(m - LAG + 1)
                            pump(950, pad=True, allow_proj=(g >= 3))
                        else:
                            pump(1900, pad=True, allow_proj=(g >= 3))
                    for m in range(MC - LAG, MC, 2):
                        emit_oa(m)
                        emit_oa(m + 1)
                        pump(400)

                    # epilogue: softmax normalization (no PE involvement)
                    cs = sb.tile([1, NB], F32, tag="cs", bufs=2)
                    nc.vector.tensor_copy(cs[:], oa[64:65, :])
                    osb = sb.tile([64, NB], F32, tag="osb", bufs=3)
                    nc.vector.tensor_copy(osb[:], oa[0:64, :])
                    rf = sb.tile([1, NB], F32, tag="rf", bufs=2)
                    nc.vector.reciprocal_approx_fast(out=rf[:], in_=cs[:])
                    rfd = drp.tile([1, NB], F32, tag="rfd", bufs=2)
                    nc.sync.dma_start(rfd[:], rf[:])
                    rbs = sb.tile([64, NB], F32, tag="rbs", bufs=2)
                    nc.sync.dma_start(rbs[:], rfd[:].partition_broadcast(64))
                    if h == 0:
                        mdst = stk[0:64, nb * NB : (nb + 1) * NB]
                    elif h == 1:
                        mdst = outT1[0:64, nb * NB : (nb + 1) * NB]
                    else:
                        mdst = outT2[0:64, nb * NB : (nb + 1) * NB]
                    nc.vector.tensor_tensor(out=mdst, in0=osb[:], in1=rbs[:], op=MULT)
                    if h == 1:
                        nc.sync.dma_start(
                            stk[64:128, nb * NB : (nb + 1) * NB],
                            outT1[0:64, nb * NB : (nb + 1) * NB],
                        )
                    if h == HPC - 1:
                        for k in range(nb * 8, nb * 8 + 8):
                            bg.append((960, proj_item(k), True))

            for _ in range(10):
                emit_pad(4)
            pump(10**9)

    nc.compile()
    return nc


def get_nc():
    global _NC_CACHE
    if _NC_CACHE is None:
        _NC_CACHE = build_nc()
    return _NC_CACHE


def make_in_maps(x, w_qkv, w_proj):
    """Shard inputs for the 8 cores: core c = 4*b + g."""
    in_maps = []
    for c in range(8):
        b, g = divmod(c, 4)
        heads = [3 * g + h for h in range(HPC)]
        xt = np.ascontiguousarray(x[b].T).astype(ml_dtypes.bfloat16)
        wqk = np.empty((C, HPC * 128), dtype=ml_dtypes.bfloat16)
        wv = np.empty((C, HPC * D), dtype=ml_dtypes.bfloat16)
        for i, hh in enumerate(heads):
            wqk[:, i * 128 : i * 128 + 64] = w_qkv[:, hh * D : (hh + 1) * D]
            wqk[:, i * 128 + 64 : i * 128 + 128] = w_qkv[
                :, C + hh * D : C + (hh + 1) * D
            ]
            wv[:, i * D : (i + 1) * D] = w_qkv[:, 2 * C + hh * D : 2 * C + (hh + 1) * D]
        wp = np.ascontiguousarray(w_proj[g * HPC * D : (g + 1) * HPC * D, :]).astype(
            ml_dtypes.bfloat16
        )
        in_maps.append(
            {"xt": xt, "wqk": np.ascontiguousarray(wqk), "wv": wv, "wp": wp}
        )
    return in_maps


def run(x, w_qkv, w_proj, b_proj, trace=False):
    nc = get_nc()
    in_maps = make_in_maps(x, w_qkv, w_proj)
    res = run_bass_kernel_spmd(nc, in_maps, core_ids=list(range(8)), trace=trace)
    out = np.empty((B, N, C), dtype=np.float32)
    for b in range(B):
        acc = res.results[4 * b]["out"].astype(np.float32)
        for g in range(1, 4):
            acc = acc + res.results[4 * b + g]["out"]
        out[b] = acc + b_proj[None, :].astype(np.float32)
    return out, res


def kernel(x, w_qkv, w_proj, b_proj):
    out, _ = run(
        np.asarray(x), np.asarray(w_qkv), np.asarray(w_proj), np.asarray(b_proj)
    )
    return out
